# revision 30
# baseline (speedup 1.0000x reference)
"""CARAFE-naive upsampling (N=4, C=256, H=W=64, k=5, g=4, s=2) on 8 TRN2
NeuronCores.

Strategy
--------
Sharding: core c <- (batch n = c//2, group-pair j = c%2). Each core owns 128
feature channels (2 of the 4 mask groups) of one batch image.

Compute: the per-pixel mask application is reformulated as TensorEngine
matmuls. For one source row r and a w-tile of 32 source columns:

    out[(g,c), (a,w,b)] += sum_{w'} statT[(g,w'), (g,c)] * B[(g,w'), (a,w,b)]

where statT is the (block-diagonal over the 2 groups) transposed feature row
and B is a *banded* matrix holding mask values on shifted diagonals
(row w+dj pairs source column w0+w+dj-2 with output column w). The 5 row
offsets di accumulate into PSUM (start/stop accumulation groups).

B cannot be built on-device (its diagonal layout is not an affine access
pattern), so the host pre-shears masks into B in numpy and ships it to HBM
in matmul-ready bf16 layout. bf16 inflation is 7.2x over raw masks but the
TensorEngine then does all 52M MACs/core in ~628 matmuls.
"""

import sys

import numpy as np

for _p in ("/opt/trn_rl_repo", "/opt/pypackages"):
    if _p not in sys.path:
        sys.path.append(_p)

import ml_dtypes  # noqa: E402
from contextlib import ExitStack  # noqa: E402

import concourse.bass as bass  # noqa: E402
import concourse.tile as tile  # noqa: E402
from concourse import bacc, mybir  # noqa: E402
from concourse.bass_utils import run_bass_kernel_spmd  # noqa: E402

# Problem constants (hardcoded per harness contract)
KS = 5            # kernel size
G = 4             # mask groups
S = 2             # upscale
N, C, H, W = 4, 256, 64, 64
Wt = 32           # w-tile
NT = W // Wt      # 2 tiles
KB = Wt + 4       # band rows per group
KK = 2 * KB       # contraction dim = 72
BF16 = ml_dtypes.bfloat16

_NC_CACHE = {}


def _build_bass():
    # Bacc (not raw Bass): its finalize() runs generate_event_semaphores,
    # which splits multi-sem waits to satisfy the 1-wait-per-instruction
    # TRN2 ISA constraint.
    nc = bacc.Bacc()
    # k-major layouts: every DMA walks contiguous bytes per SBUF partition.
    # stat rows are host-padded [72:128) with zeros: the padded contraction
    # rows of B then contribute exactly 0 (B padding stays finite).
    stat_d = nc.declare_dram_parameter(
        "stat", [128, H, NT, 128], mybir.dt.bfloat16, isOutput=False)
    # B batched 8 output rows per DMA: 8 triggers, 20KB/partition bursts.
    # Block 0 is shipped with its padding rows (bmat0) so no memset sits
    # on the critical path to the first matmul.
    BH = 8
    bmat0_d = nc.declare_dram_parameter(
        "bmat0", [128, BH, KS, NT, 128], mybir.dt.bfloat16, isOutput=False)
    bmat_d = nc.declare_dram_parameter(
        "bmat", [H // BH - 1, KK, BH, KS, NT, 128], mybir.dt.bfloat16,
        isOutput=False)
    out_d = nc.declare_dram_parameter(
        "out", [128, S * H, S * W], mybir.dt.bfloat16, isOutput=True)

    HB = 16  # output rows per batched store
    out_rows = out_d.rearrange("c (hb y) x -> c hb (y x)", hb=H // HB)

    with tile.TileContext(nc) as tc, ExitStack() as ctx:
        statp = ctx.enter_context(tc.tile_pool(name="statp", bufs=1))
        bp = ctx.enter_context(tc.tile_pool(name="bp", bufs=4))
        pp = ctx.enter_context(tc.tile_pool(name="pp", bufs=8, space="PSUM"))
        op = ctx.enter_context(tc.tile_pool(name="op", bufs=2))

        btiles = {}
        psums = {}
        otiles = {}

        def load_b(hb):
            bt = bp.tile([128, BH, KS, NT, 128], mybir.dt.bfloat16,
                         name=f"bt{hb}", tag="bt")
            if hb == 0:
                # block 0: padding rows come from DRAM (host-zeroed)
                nc.sync.dma_start(out=bt, in_=bmat0_d[:])
            else:
                # padded K rows multiply zeroed stationary rows; memset
                # keeps them finite (NaN x 0 = NaN) on the first pass
                # through a slot. Slots are reused with rows [72:128)
                # never rewritten, so the zeros persist.
                if hb < 4:
                    (nc.gpsimd if hb % 2 == 0 else nc.vector).memset(
                        bt[64:128], 0.0)
                eng = nc.gpsimd if hb % 2 == 0 else nc.sync
                eng.dma_start(out=bt[0:KK], in_=bmat_d[hb - 1])
            btiles[hb] = bt

        # B block 0 first: it gates the first matmul
        load_b(0)

        # Stationaries in SBUF; K padded 72->128 so LDWEIGHTS gets FWL
        # (needs NumWeights==128). One tile per 16-row chunk so early
        # matmuls depend only on chunk 0's DMA.
        stats = []
        for rb in range(H // 16):
            st = statp.tile([128, 16, NT, 128], mybir.dt.bfloat16,
                            name=f"st{rb}", tag=f"st{rb}")
            nc.sync.dma_start(out=st, in_=stat_d[:, 16 * rb: 16 * rb + 16])
            stats.append(st)

        for hb in range(1, 3):  # warm the next two B slots
            load_b(hb)

        def rfirst(h):
            return max(0, h - 2)

        def rlast(h):
            return min(H - 1, h + 2)

        for r in range(H):
            # prefetch B blocks two ahead of the live window
            hb_need = min(H // BH - 1, (r + 5) // BH + 1)
            if hb_need not in btiles:
                load_b(hb_need)

            # t-outer: 5 consecutive matmuls share one stationary
            for t in range(NT):
                for di in range(KS):
                    h = r + 2 - di
                    if not (0 <= h < H):
                        continue
                    if h not in psums:
                        psums[h] = pp.tile([128, NT, 128], mybir.dt.float32,
                                           name=f"ps{h}", tag="ps")
                    nc.tensor.matmul(
                        out=psums[h][:, t, :],
                        lhsT=stats[r // 16][:, r % 16, t, :],
                        rhs=btiles[h // BH][:, h % BH, di, t, :],
                        start=(r == rfirst(h) and t == 0),
                        stop=(r == rlast(h) and t == NT - 1),
                        skip_group_check=True,
                    )

            # drain finished output rows into a 16-row staging buffer
            done = [r - 2] if r - 2 >= 0 else []
            if r == H - 1:
                done += [H - 2, H - 1]
            for h in done:
                blk = h // HB
                if blk not in otiles:
                    otiles[blk] = op.tile([128, HB, S, NT, Wt, S],
                                          mybir.dt.bfloat16,
                                          name=f"ot{blk}", tag="ot")
                ot_tawb = otiles[blk][:, h % HB].rearrange(
                    "c a t w b -> c t a w b")
                ps_tawb = psums[h].rearrange(
                    "c t (a w b) -> c t a w b", a=S, b=S)
                if h % 2 == 0:
                    nc.vector.tensor_copy(out=ot_tawb, in_=ps_tawb)
                else:
                    nc.scalar.copy(out=ot_tawb, in_=ps_tawb)
                del psums[h]
                if h % BH == BH - 1:
                    del btiles[h // BH]
                if h % HB == HB - 1:
                    # 8KB/partition fully-contiguous store
                    nc.scalar.dma_start(
                        out=out_rows[:, blk], in_=otiles[blk])
                    del otiles[blk]

    nc.finalize()
    return nc


def _host_shards(features, masks):
    """Build per-core stat/bmat arrays (bf16)."""
    in_maps = []
    iw = np.arange(Wt)
    for c in range(8):
        n, j = c // 2, c % 2
        f = features[n, 128 * j: 128 * (j + 1)]        # [128, 64, 64] f32
        m = masks[n, 50 * j: 50 * j + 50]              # [50, 128, 128] f32

        # stationaries: stat[g*KB + w', r, t, g*64 + cc] = fpad[g*64+cc, r, 32t+w']
        # rows [KK:128) stay zero (K padding)
        stat = np.zeros((128, H, NT, 128), np.float32)
        fp = np.pad(f, ((0, 0), (0, 0), (2, 2)))
        for g in range(2):
            for t in range(NT):
                sl = fp[g * 64:(g + 1) * 64, :, Wt * t: Wt * t + KB]
                stat[g * KB:(g + 1) * KB, :, t, g * 64:(g + 1) * 64] = \
                    sl.transpose(2, 1, 0)

        # banded masks: B[h, g*KB + w + dj, di, t, (a,w,b)]
        M8 = m.reshape(2, KS, KS, H, S, NT, Wt, S)     # g,di,dj,h,a,t,w,b
        B2 = np.zeros((H, KS, NT, S, S, KK, Wt), np.float32)
        for g in range(2):
            for dj in range(KS):
                src = M8[g, :, dj].transpose(1, 0, 3, 2, 5, 4)  # h,di,t,a,b,w
                B2[:, :, :, :, :, g * KB + iw + dj, iw] = src
        # [hb, KK, h8, di, t, (a,w,b)] in 8-row blocks
        B3 = B2.reshape(H // 8, 8, KS, NT, S, S, KK, Wt)
        ball = np.ascontiguousarray(
            B3.transpose(0, 6, 1, 2, 3, 4, 7, 5)).reshape(
                H // 8, KK, 8, KS, NT, 128)
        bmat = ball[1:]
        bmat0 = np.zeros((128, 8, KS, NT, 128), np.float32)
        bmat0[0:KK] = ball[0]

        in_maps.append({
            "stat": np.ascontiguousarray(stat).astype(BF16),
            "bmat0": bmat0.astype(BF16),
            "bmat": np.ascontiguousarray(bmat).astype(BF16),
        })
    return in_maps


def kernel(features, masks, _trace=False):
    features = np.asarray(features, dtype=np.float32)
    masks = np.asarray(masks, dtype=np.float32)

    in_maps = _host_shards(features, masks)

    if "nc" not in _NC_CACHE:
        _NC_CACHE["nc"] = _build_bass()
    nc = _NC_CACHE["nc"]

    res = run_bass_kernel_spmd(nc, in_maps, list(range(8)), trace=_trace)
    kernel._last_result = res

    out = np.empty((N, C, S * H, S * W), np.float32)
    for c in range(8):
        n, j = c // 2, c % 2
        out[n, 128 * j: 128 * (j + 1)] = \
            res.results[c]["out"].astype(np.float32)
    return out


# revision 31
# speedup vs baseline: 1.0546x; 1.0546x over previous
"""CARAFE-naive upsampling (N=4, C=256, H=W=64, k=5, g=4, s=2) on 8 TRN2
NeuronCores.

Strategy
--------
Sharding: core c <- (batch n = c//2, group-pair j = c%2). Each core owns 128
feature channels (2 of the 4 mask groups) of one batch image.

Compute: the per-pixel mask application is reformulated as TensorEngine
matmuls. For one source row r and a w-tile of 32 source columns:

    out[(g,c), (a,w,b)] += sum_{w'} statT[(g,w'), (g,c)] * B[(g,w'), (a,w,b)]

where statT is the (block-diagonal over the 2 groups) transposed feature row
and B is a *banded* matrix holding mask values on shifted diagonals
(row w+dj pairs source column w0+w+dj-2 with output column w). The 5 row
offsets di accumulate into PSUM (start/stop accumulation groups).

B cannot be built on-device (its diagonal layout is not an affine access
pattern), so the host pre-shears masks into B in numpy and ships it to HBM
in matmul-ready bf16 layout. bf16 inflation is 7.2x over raw masks but the
TensorEngine then does all 52M MACs/core in ~628 matmuls.
"""

import sys

import numpy as np

for _p in ("/opt/trn_rl_repo", "/opt/pypackages"):
    if _p not in sys.path:
        sys.path.append(_p)

import ml_dtypes  # noqa: E402
from contextlib import ExitStack  # noqa: E402

import concourse.bass as bass  # noqa: E402
import concourse.tile as tile  # noqa: E402
from concourse import bacc, mybir  # noqa: E402
from concourse.bass_utils import run_bass_kernel_spmd  # noqa: E402

# Problem constants (hardcoded per harness contract)
KS = 5            # kernel size
G = 4             # mask groups
S = 2             # upscale
N, C, H, W = 4, 256, 64, 64
Wt = 32           # w-tile
NT = W // Wt      # 2 tiles
KB = Wt + 4       # band rows per group
KK = 2 * KB       # contraction dim = 72
BF16 = ml_dtypes.bfloat16

_NC_CACHE = {}


def _build_bass():
    # Bacc (not raw Bass): its finalize() runs generate_event_semaphores,
    # which splits multi-sem waits to satisfy the 1-wait-per-instruction
    # TRN2 ISA constraint.
    nc = bacc.Bacc()
    # k-major layouts: every DMA walks contiguous bytes per SBUF partition.
    # stat rows are host-padded [72:128) with zeros: the padded contraction
    # rows of B then contribute exactly 0 (B padding stays finite).
    stat_d = nc.declare_dram_parameter(
        "stat", [128, H, NT, 128], mybir.dt.bfloat16, isOutput=False)
    # B batched 8 output rows per DMA: 8 triggers, 20KB/partition bursts.
    # Block 0 is shipped with its padding rows (bmat0) so no memset sits
    # on the critical path to the first matmul.
    BH = 8
    bmat0_d = nc.declare_dram_parameter(
        "bmat0", [128, BH, KS, NT, 128], mybir.dt.bfloat16, isOutput=False)
    bmat_d = nc.declare_dram_parameter(
        "bmat", [H // BH - 1, KK, BH, KS, NT, 128], mybir.dt.bfloat16,
        isOutput=False)
    out_d = nc.declare_dram_parameter(
        "out", [128, S * H, S * W], mybir.dt.bfloat16, isOutput=True)

    HB = 16  # output rows per batched store
    out_rows = out_d.rearrange("c (hb y) x -> c hb (y x)", hb=H // HB)

    with tile.TileContext(nc) as tc, ExitStack() as ctx:
        statp = ctx.enter_context(tc.tile_pool(name="statp", bufs=1))
        bp = ctx.enter_context(tc.tile_pool(name="bp", bufs=4))
        pp = ctx.enter_context(tc.tile_pool(name="pp", bufs=8, space="PSUM"))
        op = ctx.enter_context(tc.tile_pool(name="op", bufs=2))

        btiles = {}
        psums = {}
        otiles = {}

        def load_b(hb):
            bt = bp.tile([128, BH, KS, NT, 128], mybir.dt.bfloat16,
                         name=f"bt{hb}", tag="bt")
            if hb == 0:
                # block 0: padding rows come from DRAM (host-zeroed)
                nc.sync.dma_start(out=bt, in_=bmat0_d[:])
            else:
                # padded K rows multiply zeroed stationary rows; memset
                # keeps them finite (NaN x 0 = NaN) on the first pass
                # through a slot. Slots are reused with rows [72:128)
                # never rewritten, so the zeros persist.
                if hb < 4:
                    (nc.gpsimd if hb % 2 == 0 else nc.vector).memset(
                        bt[64:128], 0.0)
                # HWDGE lanes only — SWDGE descriptor generation is ~1us
                # per descriptor and stalls the B feed
                eng = nc.sync if hb % 2 == 0 else nc.scalar
                eng.dma_start(out=bt[0:KK], in_=bmat_d[hb - 1])
            btiles[hb] = bt

        # B block 0 first: it gates the first matmul
        load_b(0)

        # Stationaries in SBUF; K padded 72->128 so LDWEIGHTS gets FWL
        # (needs NumWeights==128). One tile per 16-row chunk so early
        # matmuls depend only on chunk 0's DMA.
        stats = []
        for rb in range(H // 16):
            st = statp.tile([128, 16, NT, 128], mybir.dt.bfloat16,
                            name=f"st{rb}", tag=f"st{rb}")
            nc.sync.dma_start(out=st, in_=stat_d[:, 16 * rb: 16 * rb + 16])
            stats.append(st)

        for hb in range(1, 3):  # warm the next two B slots
            load_b(hb)

        def rfirst(h):
            return max(0, h - 2)

        def rlast(h):
            return min(H - 1, h + 2)

        for r in range(H):
            # prefetch B blocks two ahead of the live window
            hb_need = min(H // BH - 1, (r + 5) // BH + 1)
            if hb_need not in btiles:
                load_b(hb_need)

            # t-outer: 5 consecutive matmuls share one stationary
            for t in range(NT):
                for di in range(KS):
                    h = r + 2 - di
                    if not (0 <= h < H):
                        continue
                    if h not in psums:
                        psums[h] = pp.tile([128, NT, 128], mybir.dt.float32,
                                           name=f"ps{h}", tag="ps")
                    nc.tensor.matmul(
                        out=psums[h][:, t, :],
                        lhsT=stats[r // 16][:, r % 16, t, :],
                        rhs=btiles[h // BH][:, h % BH, di, t, :],
                        start=(r == rfirst(h) and t == 0),
                        stop=(r == rlast(h) and t == NT - 1),
                        skip_group_check=True,
                    )

            # drain finished output rows into a 16-row staging buffer
            done = [r - 2] if r - 2 >= 0 else []
            if r == H - 1:
                done += [H - 2, H - 1]
            for h in done:
                blk = h // HB
                if blk not in otiles:
                    otiles[blk] = op.tile([128, HB, S, NT, Wt, S],
                                          mybir.dt.bfloat16,
                                          name=f"ot{blk}", tag="ot")
                ot_tawb = otiles[blk][:, h % HB].rearrange(
                    "c a t w b -> c t a w b")
                ps_tawb = psums[h].rearrange(
                    "c t (a w b) -> c t a w b", a=S, b=S)
                if h % 2 == 0:
                    nc.vector.tensor_copy(out=ot_tawb, in_=ps_tawb)
                else:
                    nc.scalar.copy(out=ot_tawb, in_=ps_tawb)
                del psums[h]
                if h % BH == BH - 1:
                    del btiles[h // BH]
                if h % HB == HB - 1:
                    # 8KB/partition fully-contiguous store
                    nc.scalar.dma_start(
                        out=out_rows[:, blk], in_=otiles[blk])
                    del otiles[blk]

    nc.finalize()
    return nc


def _host_shards(features, masks):
    """Build per-core stat/bmat arrays (bf16)."""
    in_maps = []
    iw = np.arange(Wt)
    for c in range(8):
        n, j = c // 2, c % 2
        f = features[n, 128 * j: 128 * (j + 1)]        # [128, 64, 64] f32
        m = masks[n, 50 * j: 50 * j + 50]              # [50, 128, 128] f32

        # stationaries: stat[g*KB + w', r, t, g*64 + cc] = fpad[g*64+cc, r, 32t+w']
        # rows [KK:128) stay zero (K padding)
        stat = np.zeros((128, H, NT, 128), np.float32)
        fp = np.pad(f, ((0, 0), (0, 0), (2, 2)))
        for g in range(2):
            for t in range(NT):
                sl = fp[g * 64:(g + 1) * 64, :, Wt * t: Wt * t + KB]
                stat[g * KB:(g + 1) * KB, :, t, g * 64:(g + 1) * 64] = \
                    sl.transpose(2, 1, 0)

        # banded masks: B[h, g*KB + w + dj, di, t, (a,w,b)]
        M8 = m.reshape(2, KS, KS, H, S, NT, Wt, S)     # g,di,dj,h,a,t,w,b
        B2 = np.zeros((H, KS, NT, S, S, KK, Wt), np.float32)
        for g in range(2):
            for dj in range(KS):
                src = M8[g, :, dj].transpose(1, 0, 3, 2, 5, 4)  # h,di,t,a,b,w
                B2[:, :, :, :, :, g * KB + iw + dj, iw] = src
        # [hb, KK, h8, di, t, (a,w,b)] in 8-row blocks
        B3 = B2.reshape(H // 8, 8, KS, NT, S, S, KK, Wt)
        ball = np.ascontiguousarray(
            B3.transpose(0, 6, 1, 2, 3, 4, 7, 5)).reshape(
                H // 8, KK, 8, KS, NT, 128)
        bmat = ball[1:]
        bmat0 = np.zeros((128, 8, KS, NT, 128), np.float32)
        bmat0[0:KK] = ball[0]

        in_maps.append({
            "stat": np.ascontiguousarray(stat).astype(BF16),
            "bmat0": bmat0.astype(BF16),
            "bmat": np.ascontiguousarray(bmat).astype(BF16),
        })
    return in_maps


def kernel(features, masks, _trace=False):
    features = np.asarray(features, dtype=np.float32)
    masks = np.asarray(masks, dtype=np.float32)

    in_maps = _host_shards(features, masks)

    if "nc" not in _NC_CACHE:
        _NC_CACHE["nc"] = _build_bass()
    nc = _NC_CACHE["nc"]

    res = run_bass_kernel_spmd(nc, in_maps, list(range(8)), trace=_trace)
    kernel._last_result = res

    out = np.empty((N, C, S * H, S * W), np.float32)
    for c in range(8):
        n, j = c // 2, c % 2
        out[n, 128 * j: 128 * (j + 1)] = \
            res.results[c]["out"].astype(np.float32)
    return out


# revision 34
# speedup vs baseline: 1.1315x; 1.0729x over previous
"""CARAFE-naive upsampling (N=4, C=256, H=W=64, k=5, g=4, s=2) on 8 TRN2
NeuronCores.

Strategy
--------
Sharding: core c <- (batch n = c//2, group-pair j = c%2). Each core owns 128
feature channels (2 of the 4 mask groups) of one batch image.

Compute: the per-pixel mask application is reformulated as TensorEngine
matmuls. For one source row r and a w-tile of 32 source columns:

    out[(g,c), (a,w,b)] += sum_{w'} statT[(g,w'), (g,c)] * B[(g,w'), (a,w,b)]

where statT is the (block-diagonal over the 2 groups) transposed feature row
and B is a *banded* matrix holding mask values on shifted diagonals
(row w+dj pairs source column w0+w+dj-2 with output column w). The 5 row
offsets di accumulate into PSUM (start/stop accumulation groups).

B cannot be built on-device (its diagonal layout is not an affine access
pattern), so the host pre-shears masks into B in numpy and ships it to HBM
in matmul-ready bf16 layout. bf16 inflation is 7.2x over raw masks but the
TensorEngine then does all 52M MACs/core in ~628 matmuls.
"""

import sys

import numpy as np

for _p in ("/opt/trn_rl_repo", "/opt/pypackages"):
    if _p not in sys.path:
        sys.path.append(_p)

import ml_dtypes  # noqa: E402
from contextlib import ExitStack  # noqa: E402

import concourse.bass as bass  # noqa: E402
import concourse.tile as tile  # noqa: E402
from concourse import bacc, mybir  # noqa: E402
from concourse.bass_utils import run_bass_kernel_spmd  # noqa: E402

# Problem constants (hardcoded per harness contract)
KS = 5            # kernel size
G = 4             # mask groups
S = 2             # upscale
N, C, H, W = 4, 256, 64, 64
Wt = 32           # w-tile
NT = W // Wt      # 2 tiles
KB = Wt + 4       # band rows per group
KK = 2 * KB       # contraction dim = 72
BF16 = ml_dtypes.bfloat16

_NC_CACHE = {}


def _build_bass():
    # Bacc (not raw Bass): its finalize() runs generate_event_semaphores,
    # which splits multi-sem waits to satisfy the 1-wait-per-instruction
    # TRN2 ISA constraint.
    nc = bacc.Bacc()
    # k-major layouts: every DMA walks contiguous bytes per SBUF partition.
    # stat rows are host-padded [72:128) with zeros: the padded contraction
    # rows of B then contribute exactly 0 (B padding stays finite).
    stat_d = nc.declare_dram_parameter(
        "stat", [128, H, NT, 128], mybir.dt.bfloat16, isOutput=False)
    # B batched 4 output rows per DMA: fine-grained deps, 10KB bursts.
    # Block 0 is shipped with its padding rows (bmat0) so no memset sits
    # on the critical path to the first matmul.
    BH = 4
    bmat0_d = nc.declare_dram_parameter(
        "bmat0", [128, BH, KS, NT, 128], mybir.dt.bfloat16, isOutput=False)
    bmat_d = nc.declare_dram_parameter(
        "bmat", [H // BH - 1, KK, BH, KS, NT, 128], mybir.dt.bfloat16,
        isOutput=False)
    out_d = nc.declare_dram_parameter(
        "out", [128, S * H, S * W], mybir.dt.bfloat16, isOutput=True)

    HB = 16  # output rows per batched store
    out_rows = out_d.rearrange("c (hb y) x -> c hb (y x)", hb=H // HB)

    with tile.TileContext(nc) as tc, ExitStack() as ctx:
        statp = ctx.enter_context(tc.tile_pool(name="statp", bufs=1))
        bp = ctx.enter_context(tc.tile_pool(name="bp", bufs=4))
        pp = ctx.enter_context(tc.tile_pool(name="pp", bufs=8, space="PSUM"))
        op = ctx.enter_context(tc.tile_pool(name="op", bufs=2))

        btiles = {}
        psums = {}
        otiles = {}

        def load_b(hb):
            bt = bp.tile([128, BH, KS, NT, 128], mybir.dt.bfloat16,
                         name=f"bt{hb}", tag="bt")
            if hb == 0:
                # block 0: padding rows come from DRAM (host-zeroed)
                nc.sync.dma_start(out=bt, in_=bmat0_d[:])
            else:
                # padded K rows multiply zeroed stationary rows; memset
                # keeps them finite (NaN x 0 = NaN) on the first pass
                # through a slot. Slots are reused with rows [72:128)
                # never rewritten, so the zeros persist.
                if hb < 4:
                    (nc.gpsimd if hb % 2 == 0 else nc.vector).memset(
                        bt[64:128], 0.0)
                # HWDGE lanes only — SWDGE descriptor generation is ~1us
                # per descriptor and stalls the B feed
                eng = nc.sync if hb % 2 == 0 else nc.scalar
                eng.dma_start(out=bt[0:KK], in_=bmat_d[hb - 1])
            btiles[hb] = bt

        # B block 0 first: it gates the first matmul
        load_b(0)

        # Stationaries in SBUF; K padded 72->128 so LDWEIGHTS gets FWL
        # (needs NumWeights==128). One tile per 16-row chunk so early
        # matmuls depend only on chunk 0's DMA.
        stats = []
        for rb in range(H // 16):
            st = statp.tile([128, 16, NT, 128], mybir.dt.bfloat16,
                            name=f"st{rb}", tag=f"st{rb}")
            nc.sync.dma_start(out=st, in_=stat_d[:, 16 * rb: 16 * rb + 16])
            stats.append(st)

        for hb in range(1, 4):  # warm the remaining B slots
            load_b(hb)

        def rfirst(h):
            return max(0, h - 2)

        def rlast(h):
            return min(H - 1, h + 2)

        for r in range(H):
            # prefetch B blocks two ahead of the live window
            hb_need = min(H // BH - 1, (r + 5) // BH + 1)
            if hb_need not in btiles:
                load_b(hb_need)

            # t-outer: 5 consecutive matmuls share one stationary
            for t in range(NT):
                for di in range(KS):
                    h = r + 2 - di
                    if not (0 <= h < H):
                        continue
                    if h not in psums:
                        psums[h] = pp.tile([128, NT, 128], mybir.dt.float32,
                                           name=f"ps{h}", tag="ps")
                    nc.tensor.matmul(
                        out=psums[h][:, t, :],
                        lhsT=stats[r // 16][:, r % 16, t, :],
                        rhs=btiles[h // BH][:, h % BH, di, t, :],
                        start=(r == rfirst(h) and t == 0),
                        stop=(r == rlast(h) and t == NT - 1),
                        skip_group_check=True,
                    )

            # drain finished output rows into a 16-row staging buffer
            done = [r - 2] if r - 2 >= 0 else []
            if r == H - 1:
                done += [H - 2, H - 1]
            for h in done:
                blk = h // HB
                if blk not in otiles:
                    otiles[blk] = op.tile([128, HB, S, NT, Wt, S],
                                          mybir.dt.bfloat16,
                                          name=f"ot{blk}", tag="ot")
                ot_tawb = otiles[blk][:, h % HB].rearrange(
                    "c a t w b -> c t a w b")
                ps_tawb = psums[h].rearrange(
                    "c t (a w b) -> c t a w b", a=S, b=S)
                if h % 2 == 0:
                    nc.vector.tensor_copy(out=ot_tawb, in_=ps_tawb)
                else:
                    nc.scalar.copy(out=ot_tawb, in_=ps_tawb)
                del psums[h]
                if h % BH == BH - 1:
                    del btiles[h // BH]
                if h % HB == HB - 1:
                    # 8KB/partition fully-contiguous store
                    nc.scalar.dma_start(
                        out=out_rows[:, blk], in_=otiles[blk])
                    del otiles[blk]

    nc.finalize()
    return nc


def _host_shards(features, masks):
    """Build per-core stat/bmat arrays (bf16)."""
    in_maps = []
    iw = np.arange(Wt)
    for c in range(8):
        n, j = c // 2, c % 2
        f = features[n, 128 * j: 128 * (j + 1)]        # [128, 64, 64] f32
        m = masks[n, 50 * j: 50 * j + 50]              # [50, 128, 128] f32

        # stationaries: stat[g*KB + w', r, t, g*64 + cc] = fpad[g*64+cc, r, 32t+w']
        # rows [KK:128) stay zero (K padding)
        stat = np.zeros((128, H, NT, 128), np.float32)
        fp = np.pad(f, ((0, 0), (0, 0), (2, 2)))
        for g in range(2):
            for t in range(NT):
                sl = fp[g * 64:(g + 1) * 64, :, Wt * t: Wt * t + KB]
                stat[g * KB:(g + 1) * KB, :, t, g * 64:(g + 1) * 64] = \
                    sl.transpose(2, 1, 0)

        # banded masks: B[h, g*KB + w + dj, di, t, (a,w,b)]
        M8 = m.reshape(2, KS, KS, H, S, NT, Wt, S)     # g,di,dj,h,a,t,w,b
        B2 = np.zeros((H, KS, NT, S, S, KK, Wt), np.float32)
        for g in range(2):
            for dj in range(KS):
                src = M8[g, :, dj].transpose(1, 0, 3, 2, 5, 4)  # h,di,t,a,b,w
                B2[:, :, :, :, :, g * KB + iw + dj, iw] = src
        # [hb, KK, h4, di, t, (a,w,b)] in 4-row blocks
        B3 = B2.reshape(H // 4, 4, KS, NT, S, S, KK, Wt)
        ball = np.ascontiguousarray(
            B3.transpose(0, 6, 1, 2, 3, 4, 7, 5)).reshape(
                H // 4, KK, 4, KS, NT, 128)
        bmat = ball[1:]
        bmat0 = np.zeros((128, 4, KS, NT, 128), np.float32)
        bmat0[0:KK] = ball[0]

        in_maps.append({
            "stat": np.ascontiguousarray(stat).astype(BF16),
            "bmat0": bmat0.astype(BF16),
            "bmat": np.ascontiguousarray(bmat).astype(BF16),
        })
    return in_maps


def kernel(features, masks, _trace=False):
    features = np.asarray(features, dtype=np.float32)
    masks = np.asarray(masks, dtype=np.float32)

    in_maps = _host_shards(features, masks)

    if "nc" not in _NC_CACHE:
        _NC_CACHE["nc"] = _build_bass()
    nc = _NC_CACHE["nc"]

    res = run_bass_kernel_spmd(nc, in_maps, list(range(8)), trace=_trace)
    kernel._last_result = res

    out = np.empty((N, C, S * H, S * W), np.float32)
    for c in range(8):
        n, j = c // 2, c % 2
        out[n, 128 * j: 128 * (j + 1)] = \
            res.results[c]["out"].astype(np.float32)
    return out


# revision 36
# speedup vs baseline: 1.2313x; 1.0882x over previous
"""CARAFE-naive upsampling (N=4, C=256, H=W=64, k=5, g=4, s=2) on 8 TRN2
NeuronCores.

Strategy
--------
Sharding: core c <- (batch n = c//2, group-pair j = c%2). Each core owns 128
feature channels (2 of the 4 mask groups) of one batch image.

Compute: the per-pixel mask application is reformulated as TensorEngine
matmuls. For one source row r and a w-tile of 32 source columns:

    out[(g,c), (a,w,b)] += sum_{w'} statT[(g,w'), (g,c)] * B[(g,w'), (a,w,b)]

where statT is the (block-diagonal over the 2 groups) transposed feature row
and B is a *banded* matrix holding mask values on shifted diagonals
(row w+dj pairs source column w0+w+dj-2 with output column w). The 5 row
offsets di accumulate into PSUM (start/stop accumulation groups).

B cannot be built on-device (its diagonal layout is not an affine access
pattern), so the host pre-shears masks into B in numpy and ships it to HBM
in matmul-ready bf16 layout. bf16 inflation is 7.2x over raw masks but the
TensorEngine then does all 52M MACs/core in ~628 matmuls.
"""

import sys

import numpy as np

for _p in ("/opt/trn_rl_repo", "/opt/pypackages"):
    if _p not in sys.path:
        sys.path.append(_p)

import ml_dtypes  # noqa: E402
from contextlib import ExitStack  # noqa: E402

import concourse.bass as bass  # noqa: E402
import concourse.tile as tile  # noqa: E402
from concourse import bacc, mybir  # noqa: E402
from concourse.bass_utils import run_bass_kernel_spmd  # noqa: E402

# Problem constants (hardcoded per harness contract)
KS = 5            # kernel size
G = 4             # mask groups
S = 2             # upscale
N, C, H, W = 4, 256, 64, 64
Wt = 32           # w-tile
NT = W // Wt      # 2 tiles
KB = Wt + 4       # band rows per group
KK = 2 * KB       # contraction dim = 72
BF16 = ml_dtypes.bfloat16

_NC_CACHE = {}


def _build_bass():
    # Bacc (not raw Bass): its finalize() runs generate_event_semaphores,
    # which splits multi-sem waits to satisfy the 1-wait-per-instruction
    # TRN2 ISA constraint.
    nc = bacc.Bacc()
    # k-major layouts: every DMA walks contiguous bytes per SBUF partition.
    # stat rows are host-padded [72:128) with zeros: the padded contraction
    # rows of B then contribute exactly 0 (B padding stays finite).
    stat_d = nc.declare_dram_parameter(
        "stat", [128, H, NT, 128], mybir.dt.bfloat16, isOutput=False)
    # B: one tile per output row pair h — fine-grained deps pipeline best.
    # The first 4 tiles ship with padding rows (bmat0): no memset gates
    # the first matmuls.
    bmat0_d = nc.declare_dram_parameter(
        "bmat0", [4, 128, KS, NT, 128], mybir.dt.bfloat16, isOutput=False)
    bmat_d = nc.declare_dram_parameter(
        "bmat", [H - 4, KK, KS, NT, 128], mybir.dt.bfloat16, isOutput=False)
    out_d = nc.declare_dram_parameter(
        "out", [128, S * H, S * W], mybir.dt.bfloat16, isOutput=True)

    NSLOT = 12   # B tile slots
    HB = 8       # output rows per batched store
    out_rows = out_d.rearrange("c (hb y) x -> c hb (y x)", hb=H // HB)

    with tile.TileContext(nc) as tc, ExitStack() as ctx:
        statp = ctx.enter_context(tc.tile_pool(name="statp", bufs=1))
        bp = ctx.enter_context(tc.tile_pool(name="bp", bufs=NSLOT))
        pp = ctx.enter_context(tc.tile_pool(name="pp", bufs=8, space="PSUM"))
        op = ctx.enter_context(tc.tile_pool(name="op", bufs=3))

        btiles = {}
        psums = {}
        otiles = {}

        def load_b(h):
            bt = bp.tile([128, KS, NT, 128], mybir.dt.bfloat16,
                         name=f"bt{h}", tag="bt")
            if h < 4:
                # padding rows come from DRAM (host-zeroed)
                eng = nc.sync if h % 2 == 0 else nc.scalar
                eng.dma_start(out=bt, in_=bmat0_d[h])
            else:
                # padded K rows multiply zeroed stationary rows; memset
                # keeps them finite (NaN x 0 = NaN) on the first pass
                # through a slot; slots reuse rows [72:128) untouched.
                if h < NSLOT:
                    (nc.vector if h % 2 == 0 else nc.gpsimd).memset(
                        bt[64:128], 0.0)
                # HWDGE lanes only (SWDGE descriptor gen is ~1us each)
                eng = nc.sync if h % 2 == 0 else nc.scalar
                eng.dma_start(out=bt[0:KK], in_=bmat_d[h - 4])
            btiles[h] = bt

        load_b(0)
        load_b(1)

        # Stationaries in SBUF; K padded 72->128 so LDWEIGHTS gets FWL
        # (needs NumWeights==128). One tile per 8-row chunk so early
        # matmuls depend only on chunk 0's DMA.
        stats = []
        for rb in range(H // 8):
            st = statp.tile([128, 8, NT, 128], mybir.dt.bfloat16,
                            name=f"st{rb}", tag=f"st{rb}")
            nc.sync.dma_start(out=st, in_=stat_d[:, 8 * rb: 8 * rb + 8])
            stats.append(st)

        for h in range(2, NSLOT):  # warm the remaining B slots
            load_b(h)

        def rfirst(h):
            return max(0, h - 2)

        def rlast(h):
            return min(H - 1, h + 2)

        for r in range(H):
            # prefetch B tiles well ahead of the live window
            for h in range(max(0, r - 2), min(H - 1, r + 7) + 1):
                if h not in btiles:
                    load_b(h)

            # t-outer: 5 consecutive matmuls share one stationary
            for t in range(NT):
                for di in range(KS):
                    h = r + 2 - di
                    if not (0 <= h < H):
                        continue
                    if h not in psums:
                        psums[h] = pp.tile([128, NT, 128], mybir.dt.float32,
                                           name=f"ps{h}", tag="ps")
                    nc.tensor.matmul(
                        out=psums[h][:, t, :],
                        lhsT=stats[r // 8][:, r % 8, t, :],
                        rhs=btiles[h][:, di, t, :],
                        start=(r == rfirst(h) and t == 0),
                        stop=(r == rlast(h) and t == NT - 1),
                        skip_group_check=True,
                    )

            # drain finished output rows into an 8-row staging buffer
            done = [r - 2] if r - 2 >= 0 else []
            if r == H - 1:
                done += [H - 2, H - 1]
            for h in done:
                blk = h // HB
                if blk not in otiles:
                    otiles[blk] = op.tile([128, HB, S, NT, Wt, S],
                                          mybir.dt.bfloat16,
                                          name=f"ot{blk}", tag="ot")
                ot_tawb = otiles[blk][:, h % HB].rearrange(
                    "c a t w b -> c t a w b")
                ps_tawb = psums[h].rearrange(
                    "c t (a w b) -> c t a w b", a=S, b=S)
                # copies on DVE (otherwise idle); stores on ACT
                nc.vector.tensor_copy(out=ot_tawb, in_=ps_tawb)
                del psums[h], btiles[h]
                if h % HB == HB - 1:
                    # 4KB/partition fully-contiguous store
                    nc.scalar.dma_start(
                        out=out_rows[:, blk], in_=otiles[blk])
                    del otiles[blk]

    nc.finalize()
    return nc


def _host_shards(features, masks):
    """Build per-core stat/bmat arrays (bf16)."""
    in_maps = []
    iw = np.arange(Wt)
    for c in range(8):
        n, j = c // 2, c % 2
        f = features[n, 128 * j: 128 * (j + 1)]        # [128, 64, 64] f32
        m = masks[n, 50 * j: 50 * j + 50]              # [50, 128, 128] f32

        # stationaries: stat[g*KB + w', r, t, g*64 + cc] = fpad[g*64+cc, r, 32t+w']
        # rows [KK:128) stay zero (K padding)
        stat = np.zeros((128, H, NT, 128), np.float32)
        fp = np.pad(f, ((0, 0), (0, 0), (2, 2)))
        for g in range(2):
            for t in range(NT):
                sl = fp[g * 64:(g + 1) * 64, :, Wt * t: Wt * t + KB]
                stat[g * KB:(g + 1) * KB, :, t, g * 64:(g + 1) * 64] = \
                    sl.transpose(2, 1, 0)

        # banded masks: B[h, g*KB + w + dj, di, t, (a,w,b)]
        M8 = m.reshape(2, KS, KS, H, S, NT, Wt, S)     # g,di,dj,h,a,t,w,b
        B2 = np.zeros((H, KS, NT, S, S, KK, Wt), np.float32)
        for g in range(2):
            for dj in range(KS):
                src = M8[g, :, dj].transpose(1, 0, 3, 2, 5, 4)  # h,di,t,a,b,w
                B2[:, :, :, :, :, g * KB + iw + dj, iw] = src
        # [h, KK, di, t, (a,w,b)] per-row tiles
        ball = np.ascontiguousarray(
            B2.transpose(0, 5, 1, 2, 3, 6, 4)).reshape(H, KK, KS, NT, 128)
        bmat = ball[4:]
        bmat0 = np.zeros((4, 128, KS, NT, 128), np.float32)
        bmat0[:, 0:KK] = ball[:4]

        in_maps.append({
            "stat": np.ascontiguousarray(stat).astype(BF16),
            "bmat0": bmat0.astype(BF16),
            "bmat": np.ascontiguousarray(bmat).astype(BF16),
        })
    return in_maps


def kernel(features, masks, _trace=False):
    features = np.asarray(features, dtype=np.float32)
    masks = np.asarray(masks, dtype=np.float32)

    in_maps = _host_shards(features, masks)

    if "nc" not in _NC_CACHE:
        _NC_CACHE["nc"] = _build_bass()
    nc = _NC_CACHE["nc"]

    res = run_bass_kernel_spmd(nc, in_maps, list(range(8)), trace=_trace)
    kernel._last_result = res

    out = np.empty((N, C, S * H, S * W), np.float32)
    for c in range(8):
        n, j = c // 2, c % 2
        out[n, 128 * j: 128 * (j + 1)] = \
            res.results[c]["out"].astype(np.float32)
    return out


# revision 38
# speedup vs baseline: 1.3112x; 1.0649x over previous
"""CARAFE-naive upsampling (N=4, C=256, H=W=64, k=5, g=4, s=2) on 8 TRN2
NeuronCores.

Strategy
--------
Sharding: core c <- (batch n = c//2, group-pair j = c%2). Each core owns 128
feature channels (2 of the 4 mask groups) of one batch image.

Compute: the per-pixel mask application is reformulated as TensorEngine
matmuls. For one source row r and a w-tile of 32 source columns:

    out[(g,c), (a,w,b)] += sum_{w'} statT[(g,w'), (g,c)] * B[(g,w'), (a,w,b)]

where statT is the (block-diagonal over the 2 groups) transposed feature row
and B is a *banded* matrix holding mask values on shifted diagonals
(row w+dj pairs source column w0+w+dj-2 with output column w). The 5 row
offsets di accumulate into PSUM (start/stop accumulation groups).

B cannot be built on-device (its diagonal layout is not an affine access
pattern), so the host pre-shears masks into B in numpy and ships it to HBM
in matmul-ready bf16 layout. bf16 inflation is 7.2x over raw masks but the
TensorEngine then does all 52M MACs/core in ~628 matmuls.
"""

import sys

import numpy as np

for _p in ("/opt/trn_rl_repo", "/opt/pypackages"):
    if _p not in sys.path:
        sys.path.append(_p)

import ml_dtypes  # noqa: E402
from contextlib import ExitStack  # noqa: E402

import concourse.bass as bass  # noqa: E402
import concourse.tile as tile  # noqa: E402
from concourse import bacc, mybir  # noqa: E402
from concourse.bass_utils import run_bass_kernel_spmd  # noqa: E402

# Problem constants (hardcoded per harness contract)
KS = 5            # kernel size
G = 4             # mask groups
S = 2             # upscale
N, C, H, W = 4, 256, 64, 64
Wt = 32           # w-tile
NT = W // Wt      # 2 tiles
KB = Wt + 4       # band rows per group
KK = 2 * KB       # contraction dim = 72
BF16 = ml_dtypes.bfloat16

_NC_CACHE = {}


def _build_bass():
    # Bacc (not raw Bass): its finalize() runs generate_event_semaphores,
    # which splits multi-sem waits to satisfy the 1-wait-per-instruction
    # TRN2 ISA constraint.
    nc = bacc.Bacc()
    # k-major layouts: every DMA walks contiguous bytes per SBUF partition.
    # stat rows are host-padded [72:128) with zeros: the padded contraction
    # rows of B then contribute exactly 0 (B padding stays finite).
    stat_d = nc.declare_dram_parameter(
        "stat", [128, H, NT, 128], mybir.dt.bfloat16, isOutput=False)
    # B: one tile per output row pair h — fine-grained deps pipeline best.
    # The first 4 tiles ship with padding rows (bmat0): no memset gates
    # the first matmuls.
    bmat0_d = nc.declare_dram_parameter(
        "bmat0", [4, 128, KS, NT, 128], mybir.dt.bfloat16, isOutput=False)
    bmat_d = nc.declare_dram_parameter(
        "bmat", [H - 4, KK, KS, NT, 128], mybir.dt.bfloat16, isOutput=False)
    out_d = nc.declare_dram_parameter(
        "out", [128, S * H, S * W], mybir.dt.bfloat16, isOutput=True)

    NSLOT = 14   # B tile slots
    HB = 8       # output rows per batched store
    out_rows = out_d.rearrange("c (hb y) x -> c hb (y x)", hb=H // HB)

    with tile.TileContext(nc) as tc, ExitStack() as ctx:
        statp = ctx.enter_context(tc.tile_pool(name="statp", bufs=1))
        bp = ctx.enter_context(tc.tile_pool(name="bp", bufs=NSLOT))
        pp = ctx.enter_context(tc.tile_pool(name="pp", bufs=8, space="PSUM"))
        op = ctx.enter_context(tc.tile_pool(name="op", bufs=3))

        btiles = {}
        psums = {}
        otiles = {}

        def load_b(h):
            bt = bp.tile([128, KS, NT, 128], mybir.dt.bfloat16,
                         name=f"bt{h}", tag="bt")
            if h < 4:
                # padding rows come from DRAM (host-zeroed)
                eng = nc.sync if h % 2 == 0 else nc.scalar
                eng.dma_start(out=bt, in_=bmat0_d[h])
            else:
                # padded K rows multiply zeroed stationary rows; memset
                # keeps them finite (NaN x 0 = NaN) on the first pass
                # through a slot; slots reuse rows [72:128) untouched.
                if h < NSLOT:
                    (nc.vector if h % 2 == 0 else nc.gpsimd).memset(
                        bt[64:128], 0.0)
                # HWDGE lanes only (SWDGE descriptor gen is ~1us each)
                eng = nc.sync if h % 2 == 0 else nc.scalar
                eng.dma_start(out=bt[0:KK], in_=bmat_d[h - 4])
            btiles[h] = bt

        # Stationaries in SBUF; K padded 72->128 so LDWEIGHTS gets FWL
        # (needs NumWeights==128). 8-row chunks, loaded lazily in
        # first-use order: HW DMA queues are FIFOs, so anything emitted
        # ahead of a tile delays every consumer of that tile.
        stats = [None] * (H // 8)

        def load_stat(rb, eng):
            st = statp.tile([128, 8, NT, 128], mybir.dt.bfloat16,
                            name=f"st{rb}", tag=f"st{rb}")
            eng.dma_start(out=st, in_=stat_d[:, 8 * rb: 8 * rb + 8])
            stats[rb] = st

        load_b(0)
        load_b(1)
        load_b(2)
        load_stat(0, nc.scalar)
        for h in range(3, NSLOT):  # warm the remaining B slots
            load_b(h)
        load_stat(1, nc.sync)

        def rfirst(h):
            return max(0, h - 2)

        def rlast(h):
            return min(H - 1, h + 2)

        for r in range(H):
            # prefetch B tiles well ahead of the live window
            for h in range(max(0, r - 2), min(H - 1, r + 9) + 1):
                if h not in btiles:
                    load_b(h)
            # stat chunk for rows [8rb, 8rb+8) emitted ~10 rows ahead
            rb_need = min(H // 8 - 1, (r + 10) // 8)
            if stats[rb_need] is None:
                load_stat(rb_need, nc.sync if rb_need % 2 else nc.scalar)

            # t-outer: 5 consecutive matmuls share one stationary
            for t in range(NT):
                for di in range(KS):
                    h = r + 2 - di
                    if not (0 <= h < H):
                        continue
                    if h not in psums:
                        psums[h] = pp.tile([128, NT, 128], mybir.dt.float32,
                                           name=f"ps{h}", tag="ps")
                    nc.tensor.matmul(
                        out=psums[h][:, t, :],
                        lhsT=stats[r // 8][:, r % 8, t, :],
                        rhs=btiles[h][:, di, t, :],
                        start=(r == rfirst(h) and t == 0),
                        stop=(r == rlast(h) and t == NT - 1),
                        skip_group_check=True,
                    )

            # drain finished output rows into an 8-row staging buffer
            done = [r - 2] if r - 2 >= 0 else []
            if r == H - 1:
                done += [H - 2, H - 1]
            for h in done:
                blk = h // HB
                if blk not in otiles:
                    otiles[blk] = op.tile([128, HB, S, NT, Wt, S],
                                          mybir.dt.bfloat16,
                                          name=f"ot{blk}", tag="ot")
                ot_tawb = otiles[blk][:, h % HB].rearrange(
                    "c a t w b -> c t a w b")
                ps_tawb = psums[h].rearrange(
                    "c t (a w b) -> c t a w b", a=S, b=S)
                # copies on DVE (otherwise idle); stores on ACT
                nc.vector.tensor_copy(out=ot_tawb, in_=ps_tawb)
                del psums[h], btiles[h]
                if h % HB == HB - 1:
                    # 4KB/partition fully-contiguous store
                    nc.scalar.dma_start(
                        out=out_rows[:, blk], in_=otiles[blk])
                    del otiles[blk]

    nc.finalize()
    return nc


def _host_shards(features, masks):
    """Build per-core stat/bmat arrays (bf16)."""
    in_maps = []
    iw = np.arange(Wt)
    for c in range(8):
        n, j = c // 2, c % 2
        f = features[n, 128 * j: 128 * (j + 1)]        # [128, 64, 64] f32
        m = masks[n, 50 * j: 50 * j + 50]              # [50, 128, 128] f32

        # stationaries: stat[g*KB + w', r, t, g*64 + cc] = fpad[g*64+cc, r, 32t+w']
        # rows [KK:128) stay zero (K padding)
        stat = np.zeros((128, H, NT, 128), np.float32)
        fp = np.pad(f, ((0, 0), (0, 0), (2, 2)))
        for g in range(2):
            for t in range(NT):
                sl = fp[g * 64:(g + 1) * 64, :, Wt * t: Wt * t + KB]
                stat[g * KB:(g + 1) * KB, :, t, g * 64:(g + 1) * 64] = \
                    sl.transpose(2, 1, 0)

        # banded masks: B[h, g*KB + w + dj, di, t, (a,w,b)]
        M8 = m.reshape(2, KS, KS, H, S, NT, Wt, S)     # g,di,dj,h,a,t,w,b
        B2 = np.zeros((H, KS, NT, S, S, KK, Wt), np.float32)
        for g in range(2):
            for dj in range(KS):
                src = M8[g, :, dj].transpose(1, 0, 3, 2, 5, 4)  # h,di,t,a,b,w
                B2[:, :, :, :, :, g * KB + iw + dj, iw] = src
        # [h, KK, di, t, (a,w,b)] per-row tiles
        ball = np.ascontiguousarray(
            B2.transpose(0, 5, 1, 2, 3, 6, 4)).reshape(H, KK, KS, NT, 128)
        bmat = ball[4:]
        bmat0 = np.zeros((4, 128, KS, NT, 128), np.float32)
        bmat0[:, 0:KK] = ball[:4]

        in_maps.append({
            "stat": np.ascontiguousarray(stat).astype(BF16),
            "bmat0": bmat0.astype(BF16),
            "bmat": np.ascontiguousarray(bmat).astype(BF16),
        })
    return in_maps


def kernel(features, masks, _trace=False):
    features = np.asarray(features, dtype=np.float32)
    masks = np.asarray(masks, dtype=np.float32)

    in_maps = _host_shards(features, masks)

    if "nc" not in _NC_CACHE:
        _NC_CACHE["nc"] = _build_bass()
    nc = _NC_CACHE["nc"]

    res = run_bass_kernel_spmd(nc, in_maps, list(range(8)), trace=_trace)
    kernel._last_result = res

    out = np.empty((N, C, S * H, S * W), np.float32)
    for c in range(8):
        n, j = c // 2, c % 2
        out[n, 128 * j: 128 * (j + 1)] = \
            res.results[c]["out"].astype(np.float32)
    return out
